# revision 16
# baseline (speedup 1.0000x reference)
"""AttentionMemory decode step on 8 TRN2 NeuronCores.

Batch-parallel sharding (data parallel): core i owns batches {2i, 2i+1};
the memory bank is replicated. Each core streams its two batches' KV
cache plus the full bank through SBUF once: cache tiles are copied back
out to the updated-cache outputs (the scatter/append), key tiles are
transposed on the PE (identity matmul) to feed the score matmuls
(scoresT layout [n, b*h]), exp on the scalar engine, then the softmax
weights act as the stationary operand for the value product
(out [b*h, hvd] accumulated in PSUM) with the denominator as a free
extra N=1 matmul against a ones column. No cross-core communication.
Softmax runs without max-subtraction (scores are O(1) for this module's
scales), which is mathematically identical.
"""

import ml_dtypes
import numpy as np

import concourse.bass as bass
import concourse.bacc as bacc
import concourse.tile as tile
import concourse.mybir as mybir
from concourse.bass_utils import run_bass_kernel_spmd

B, D, M, H, KD, VD, L = 16, 1024, 8192, 8, 64, 64, 8192
F = H * KD  # 512
NCORES = 8
BL = B // NCORES  # batches per core (2)
CC = BL * H  # local score columns (16)
RM = 1024  # rows per macro tile
T = RM // 128  # 128-row tiles per macro
FP32 = mybir.dt.float32
BF16 = mybir.dt.bfloat16
Exp = mybir.ActivationFunctionType.Exp


def _emit(nc, tc):
    # ---------------- DRAM parameters ----------------
    def din(name, shape):
        return nc.dram_tensor(name, shape, FP32, kind="ExternalInput").ap()

    def dout(name, shape):
        return nc.dram_tensor(name, shape, FP32, kind="ExternalOutput").ap()

    x_d = din("x", [BL, D])
    wq_d = din("wq", [D, F])  # pre-scaled by 1/sqrt(KD) on host
    bq_d = din("bq", [1, F])  # pre-scaled
    wk_d = din("wk", [D, F])
    bk_d = din("bk", [1, F])
    wv_d = din("wv", [D, F])
    bv_d = din("bv", [1, F])
    wo_d = din("wo", [F, D])
    bo_d = din("bo", [1, D])
    # bank keys only in transpose-blocked layout [macro, chunk, f', t, n]
    mkt_d = nc.dram_tensor("mkt", [M // RM, 4, 128, T, 128], BF16, kind="ExternalInput").ap()
    mv_d = nc.dram_tensor("mv", [M, F], BF16, kind="ExternalInput").ap()
    pk_d = din("pk", [BL, L, F])
    pkt_d = din("pkt", [BL, L // RM, 4, 128, T, 128])
    pv_d = din("pv", [BL, L, F])
    id_d = din("ident", [128, 128])
    on_d = din("ones", [128, 128])

    out_d = dout("out", [BL, D])
    uk_d = dout("uk", [BL, L, F])
    uv_d = dout("uv", [BL, L, F])
    nk_d = dout("newk", [BL, F])
    nv_d = dout("newv", [BL, F])

    with (
        tc.tile_pool(name="cst", bufs=1) as cst,
        tc.tile_pool(name="sb", bufs=1) as sb,
    ):
        ident = cst.tile([128, 128], FP32)
        nc.sync.dma_start(ident[:], id_d[:])
        ones = cst.tile([128, 128], FP32)
        nc.sync.dma_start(ones[:], on_d[:])
        wo_sb = cst.tile([128, 4, D], FP32)
        nc.scalar.dma_start(wo_sb[:], wo_d.rearrange("(c p) f -> p c f", p=128))
        bo_sb = cst.tile([1, D], FP32)
        nc.scalar.dma_start(bo_sb[:], bo_d[:])

        # ------------- prologue: projections -------------
        with (
            tc.tile_pool(name="wts", bufs=1) as wts,
            tc.tile_pool(name="pps", bufs=2, space="PSUM") as pps,
        ):
            x_sb = wts.tile([BL, D], FP32)
            nc.sync.dma_start(x_sb[:], x_d[:])

            # xT: [D, BL] as 8 chunks of [128, BL]
            xt = cst.tile([128, 8 * BL], FP32)
            for c in range(8):
                xt_ps = pps.tile([128, BL], FP32, tag="xt_ps")
                nc.tensor.transpose(
                    xt_ps[:], x_sb[:, c * 128 : (c + 1) * 128], ident[0:BL, 0:BL]
                )
                nc.vector.tensor_copy(xt[:, c * BL : (c + 1) * BL], xt_ps[:])

            projs = []
            for nm, w_d, b_d in (
                ("q", wq_d, bq_d),
                ("nk", wk_d, bk_d),
                ("nv", wv_d, bv_d),
            ):
                w_sb = wts.tile([128, 8, F], FP32, tag=f"w_{nm}")
                nc.scalar.dma_start(w_sb[:], w_d.rearrange("(c p) f -> p c f", p=128))
                b_sb = wts.tile([1, F], FP32, tag=f"b_{nm}")
                nc.scalar.dma_start(b_sb[:], b_d[:])
                ps = pps.tile([BL, F], FP32, tag="proj_ps")
                for c in range(8):
                    nc.tensor.matmul(
                        ps[:],
                        xt[:, c * BL : (c + 1) * BL],
                        w_sb[:, c, :],
                        start=(c == 0),
                        stop=False,
                    )
                nc.tensor.matmul(
                    ps[:], ones[0:1, 0:BL], b_sb[:], start=False, stop=True
                )
                r_sb = cst.tile([BL, F], FP32, tag=f"r_{nm}")
                nc.vector.tensor_copy(r_sb[:], ps[:])
                projs.append(r_sb)
            q_sb, nk_sb, nv_sb = projs
            nc.scalar.dma_start(nk_d[:], nk_sb[:])
            nc.scalar.dma_start(nv_d[:], nv_sb[:])

            # qT chunks then block-diagonal qtb [128, 4*CC]
            qt = cst.tile([128, 4 * BL], FP32)
            for c in range(4):
                qt_ps = pps.tile([128, BL], FP32, tag="xt_ps")
                nc.tensor.transpose(
                    qt_ps[:], q_sb[:, c * 128 : (c + 1) * 128], ident[0:BL, 0:BL]
                )
                nc.vector.tensor_copy(qt[:, c * BL : (c + 1) * BL], qt_ps[:])

            qtb = cst.tile([128, 4 * CC], FP32)
            nc.gpsimd.memset(qtb[:], 0.0)
            qtb_v = qtb.rearrange("p (c b h) -> p c b h", c=4, b=BL, h=H)
            qt_v = qt.rearrange("p (c b) -> p c b", c=4)
            for c in range(4):
                for hh in range(2):
                    h = 2 * c + hh
                    nc.vector.tensor_copy(
                        qtb_v[64 * hh : 64 * hh + 64, c, :, h],
                        qt_v[64 * hh : 64 * hh + 64, c, :],
                    )

        # ------------- main loop -------------
        # Per 512-row macro: PE-transpose K into chunk-major KT [f',n],
        # scores [cc, 512] via 4 fat matmuls (stationary = tiny Q blocks),
        # exp once, PE-transpose the weights [cc,128]->[128,cc] per tile,
        # then weights act as stationary for the V product (N=512).
        macros = [("bank", None, j) for j in range(M // RM)]
        macros += [("cache", b, j) for b in range(BL) for j in range(L // RM)]
        with (
            tc.tile_pool(name="acc", bufs=1, space="PSUM") as accp,
            tc.tile_pool(name="mmp", bufs=2, space="PSUM") as mmp,
            tc.tile_pool(name="kv", bufs=3) as kvp,
            tc.tile_pool(name="wrk", bufs=2) as wrk,
        ):
            retr_ps = accp.tile([CC, F], FP32)
            den_ps = accp.tile([CC, 1], FP32)
            zeros = cst.tile([128, H], FP32)
            nc.gpsimd.memset(zeros[:], 0.0)
            qtb_bf = cst.tile([128, 4 * CC], BF16)
            nc.vector.tensor_copy(qtb_bf[:], qtb[:])
            ones_bf = cst.tile([128, 1], BF16)
            nc.vector.tensor_copy(ones_bf[:], ones[:, 0:1])

            for kind, b, j in macros:
                if kind == "bank":
                    ktsrc = mkt_d[j]
                    vsrc = mv_d[j * RM : (j + 1) * RM, :]
                    ncc, coff = CC, 0
                else:
                    ktsrc = pkt_d[b, j]
                    vsrc = pv_d[b, j * RM : (j + 1) * RM, :]
                    ncc, coff = H, b * H
                kdt = BF16 if kind == "bank" else FP32
                kt_sb = kvp.tile([128, 4, T, 128], kdt, tag="kt_sb")
                nc.sync.dma_start(
                    kt_sb[:], ktsrc.rearrange("c p t n -> p c t n")
                )
                vmac = kvp.tile([128, T, F], kdt, tag="vmac")
                nc.sync.dma_start(vmac[:], vsrc.rearrange("(t p) f -> p t f", p=128))
                if kind == "cache":
                    kmac = kvp.tile([128, T, F], FP32, tag="kmac")
                    nc.sync.dma_start(
                        kmac[:],
                        pk_d[b, j * RM : (j + 1) * RM, :].rearrange(
                            "(t p) f -> p t f", p=128
                        ),
                    )
                    nc.scalar.dma_start(
                        uk_d[b, j * RM : (j + 1) * RM, :].rearrange(
                            "(t p) f -> p t f", p=128
                        ),
                        kmac[:],
                    )
                    nc.scalar.dma_start(
                        uv_d[b, j * RM : (j + 1) * RM, :].rearrange(
                            "(t p) f -> p t f", p=128
                        ),
                        vmac[:],
                    )
                first = kind == "bank" and j == 0
                last = kind == "cache" and j == (L // RM) - 1

                # scores [ncc, RM] over the whole macro (two 512-wide halves)
                st_ps = mmp.tile([CC, RM], FP32, tag="st_ps")
                for c in range(4):
                    for hf in range(2):
                        nc.tensor.matmul(
                            st_ps[0:ncc, hf * 512 : (hf + 1) * 512],
                            (qtb_bf if kind == "bank" else qtb)[
                                :, c * CC + coff : c * CC + coff + ncc
                            ],
                            kt_sb[:, c, hf * 4 : (hf + 1) * 4, :],
                            start=(c == 0),
                            stop=(c == 3),
                        )
                w_row = wrk.tile([CC, RM], FP32, tag="w_row")
                nc.scalar.activation(w_row[0:ncc, :], st_ps[0:ncc, :], Exp)

                # transpose weights per 128-tile: [ncc,128] -> [128,ncc]
                wt_ps = mmp.tile([128, T * CC], FP32, tag="wt_ps")
                for t in range(T):
                    nc.tensor.transpose(
                        wt_ps[:, t * CC + coff : t * CC + coff + ncc],
                        w_row[0:ncc, t * 128 : (t + 1) * 128],
                        ident[0:ncc, 0:ncc],
                    )
                    if kind == "cache":
                        off = H - coff
                        nc.vector.tensor_copy(
                            wt_ps[:, t * CC + off : t * CC + off + H], zeros[:]
                        )
                wt_sb = wrk.tile([128, T * CC], kdt, tag="wt_sb")
                nc.vector.tensor_copy(wt_sb[:], wt_ps[:])

                st_first = first
                st_last = last
                for t in range(T):
                    nc.tensor.matmul(
                        retr_ps[:],
                        wt_sb[:, t * CC : (t + 1) * CC],
                        vmac[:, t, :],
                        start=(st_first and t == 0),
                        stop=(st_last and t == T - 1),
                        skip_group_check=True,
                    )
                    nc.tensor.matmul(
                        den_ps[:],
                        wt_sb[:, t * CC : (t + 1) * CC],
                        ones_bf[:] if kind == "bank" else ones[:, 0:1],
                        start=(st_first and t == 0),
                        stop=(st_last and t == T - 1),
                        skip_group_check=True,
                    )

            # normalize while accumulators are live: rn = retr / den
            recp = sb.tile([CC, 1], FP32)
            nc.vector.reciprocal(recp[:], den_ps[:])
            rn = sb.tile([CC, F], FP32)
            nc.vector.tensor_scalar_mul(rn[:], retr_ps[:], recp[:])

        # ------------- epilogue -------------
        with tc.tile_pool(name="eps", bufs=2, space="PSUM") as eps:
            # transpose rn [CC, F] -> rnT chunks [128, CC]
            rnt = sb.tile([128, 4 * CC], FP32)
            for c in range(4):
                rnt_ps = eps.tile([128, CC], FP32, tag="rnt_ps")
                nc.tensor.transpose(
                    rnt_ps[:], rn[:, c * 128 : (c + 1) * 128], ident[0:CC, 0:CC]
                )
                nc.vector.tensor_copy(rnt[:, c * CC : (c + 1) * CC], rnt_ps[:])

            # extract diagonal head blocks: rtn[(c f'), b] = rnT[(c f'), b*8+h(f')]
            rtn = sb.tile([128, 4 * BL], FP32)
            rtn_v = rtn.rearrange("p (c b) -> p c b", c=4)
            rnt_v = rnt.rearrange("p (c b h) -> p c b h", c=4, b=BL, h=H)
            for c in range(4):
                for hh in range(2):
                    h = 2 * c + hh
                    sl = slice(64 * hh, 64 * hh + 64)
                    nc.vector.tensor_copy(
                        rtn_v[sl, c, :], rnt_v[sl, c, :, h]
                    )

            o_sb = sb.tile([BL, D], FP32)
            for half in range(2):
                op_ps = eps.tile([BL, 512], FP32, tag="op")
                for c in range(4):
                    nc.tensor.matmul(
                        op_ps[:],
                        rtn_v[:, c, :],
                        wo_sb[:, c, half * 512 : (half + 1) * 512],
                        start=(c == 0),
                        stop=False,
                    )
                nc.tensor.matmul(
                    op_ps[:],
                    ones[0:1, 0:BL],
                    bo_sb[0:1, half * 512 : (half + 1) * 512],
                    start=False,
                    stop=True,
                )
                nc.vector.tensor_copy(o_sb[:, half * 512 : (half + 1) * 512], op_ps[:])
            nc.sync.dma_start(out_d[:], o_sb[:])


_NC_CACHE = {}


def _get_nc():
    if "nc" not in _NC_CACHE:
        nc = bacc.Bacc(
            "TRN2", target_bir_lowering=False, debug=False, num_devices=NCORES
        )
        with tile.TileContext(nc) as tc:
            _emit(nc, tc)
        nc.compile()
        _NC_CACHE["nc"] = nc
    return _NC_CACHE["nc"]


def _make_in_maps(
    x, prev_keys, prev_values, Wq, bq, Wk, bk, Wv, bv, Wo, bo,
    memory_keys, memory_values,
):
    f32 = np.float32
    x = np.ascontiguousarray(np.asarray(x, f32))
    pk = np.asarray(prev_keys, f32).reshape(B, L, F)
    pv = np.asarray(prev_values, f32).reshape(B, L, F)
    scale = np.float32(1.0 / np.sqrt(KD))
    rep = {
        "wq": np.ascontiguousarray(np.asarray(Wq, f32) * scale),
        "bq": np.asarray(bq, f32).reshape(1, F) * scale,
        "wk": np.ascontiguousarray(np.asarray(Wk, f32)),
        "bk": np.asarray(bk, f32).reshape(1, F),
        "wv": np.ascontiguousarray(np.asarray(Wv, f32)),
        "bv": np.asarray(bv, f32).reshape(1, F),
        "wo": np.ascontiguousarray(np.asarray(Wo, f32)),
        "bo": np.asarray(bo, f32).reshape(1, D),
        "mkt": np.ascontiguousarray(
            np.asarray(memory_keys, f32)
            .reshape(M // RM, T, 128, 4, 128)
            .transpose(0, 3, 4, 1, 2)
        ).astype(ml_dtypes.bfloat16),
        "mv": np.asarray(memory_values, f32).astype(ml_dtypes.bfloat16),
        "ident": np.eye(128, dtype=f32),
        "ones": np.ones((128, 128), f32),
    }
    pkt = np.ascontiguousarray(
        pk.reshape(B, L // RM, T, 128, 4, 128).transpose(0, 1, 4, 5, 2, 3)
    )
    in_maps = []
    for i in range(NCORES):
        m = dict(rep)
        m["x"] = x[i * BL : (i + 1) * BL]
        m["pk"] = pk[i * BL : (i + 1) * BL]
        m["pkt"] = pkt[i * BL : (i + 1) * BL]
        m["pv"] = pv[i * BL : (i + 1) * BL]
        in_maps.append(m)
    return in_maps


def _assemble(results):
    f32 = np.float32
    output = np.concatenate([np.asarray(r["out"]) for r in results], axis=0)
    upd_k = np.empty((B, L + 1, H, KD), f32)
    upd_v = np.empty((B, L + 1, H, VD), f32)
    for i in range(NCORES):
        bs = slice(i * BL, (i + 1) * BL)
        upd_k[bs, 0:L] = np.asarray(results[i]["uk"]).reshape(BL, L, H, KD)
        upd_v[bs, 0:L] = np.asarray(results[i]["uv"]).reshape(BL, L, H, VD)
        upd_k[bs, L] = np.asarray(results[i]["newk"]).reshape(BL, H, KD)
        upd_v[bs, L] = np.asarray(results[i]["newv"]).reshape(BL, H, VD)
    return output, upd_k, upd_v


def kernel(**inputs):
    in_maps = _make_in_maps(**inputs)
    nc = _get_nc()
    res = run_bass_kernel_spmd(nc, in_maps, list(range(NCORES)))
    _NC_CACHE["last_results"] = res
    return _assemble(res.results)


# revision 17
# speedup vs baseline: 1.0369x; 1.0369x over previous
"""AttentionMemory decode step on 8 TRN2 NeuronCores.

Batch-parallel sharding (data parallel): core i owns batches {2i, 2i+1};
the memory bank is replicated. Each core streams its two batches' KV
cache plus the full bank through SBUF once: cache tiles are copied back
out to the updated-cache outputs (the scatter/append), key tiles are
transposed on the PE (identity matmul) to feed the score matmuls
(scoresT layout [n, b*h]), exp on the scalar engine, then the softmax
weights act as the stationary operand for the value product
(out [b*h, hvd] accumulated in PSUM) with the denominator as a free
extra N=1 matmul against a ones column. No cross-core communication.
Softmax runs without max-subtraction (scores are O(1) for this module's
scales), which is mathematically identical.
"""

import numpy as np

import concourse.bass as bass
import concourse.bacc as bacc
import concourse.tile as tile
import concourse.mybir as mybir
from concourse.bass_utils import run_bass_kernel_spmd

B, D, M, H, KD, VD, L = 16, 1024, 8192, 8, 64, 64, 8192
F = H * KD  # 512
NCORES = 8
BL = B // NCORES  # batches per core (2)
CC = BL * H  # local score columns (16)
RM = 1024  # rows per macro tile
T = RM // 128  # 128-row tiles per macro
FP32 = mybir.dt.float32
BF16 = mybir.dt.bfloat16
Exp = mybir.ActivationFunctionType.Exp


def _emit(nc, tc):
    # ---------------- DRAM parameters ----------------
    def din(name, shape):
        return nc.dram_tensor(name, shape, FP32, kind="ExternalInput").ap()

    def dout(name, shape):
        return nc.dram_tensor(name, shape, FP32, kind="ExternalOutput").ap()

    x_d = din("x", [BL, D])
    wq_d = din("wq", [D, F])  # pre-scaled by 1/sqrt(KD) on host
    bq_d = din("bq", [1, F])  # pre-scaled
    wk_d = din("wk", [D, F])
    bk_d = din("bk", [1, F])
    wv_d = din("wv", [D, F])
    bv_d = din("bv", [1, F])
    wo_d = din("wo", [F, D])
    bo_d = din("bo", [1, D])
    # bank keys only in transpose-blocked layout [macro, chunk, f', t, n]
    mkt_d = din("mkt", [M // RM, 4, 128, T, 128])
    mv_d = din("mv", [M, F])
    pk_d = din("pk", [BL, L, F])
    pkt_d = din("pkt", [BL, L // RM, 4, 128, T, 128])
    pv_d = din("pv", [BL, L, F])
    id_d = din("ident", [128, 128])
    on_d = din("ones", [128, 128])

    out_d = dout("out", [BL, D])
    uk_d = dout("uk", [BL, L, F])
    uv_d = dout("uv", [BL, L, F])
    nk_d = dout("newk", [BL, F])
    nv_d = dout("newv", [BL, F])

    with (
        tc.tile_pool(name="cst", bufs=1) as cst,
        tc.tile_pool(name="sb", bufs=1) as sb,
    ):
        ident = cst.tile([128, 128], FP32)
        nc.sync.dma_start(ident[:], id_d[:])
        ones = cst.tile([128, 128], FP32)
        nc.sync.dma_start(ones[:], on_d[:])
        wo_sb = cst.tile([128, 4, D], FP32)
        nc.scalar.dma_start(wo_sb[:], wo_d.rearrange("(c p) f -> p c f", p=128))
        bo_sb = cst.tile([1, D], FP32)
        nc.scalar.dma_start(bo_sb[:], bo_d[:])

        # ------------- prologue: projections -------------
        with (
            tc.tile_pool(name="wts", bufs=1) as wts,
            tc.tile_pool(name="pps", bufs=2, space="PSUM") as pps,
        ):
            x_sb = wts.tile([BL, D], FP32)
            nc.sync.dma_start(x_sb[:], x_d[:])

            # xT: [D, BL] as 8 chunks of [128, BL]
            xt = cst.tile([128, 8 * BL], FP32)
            for c in range(8):
                xt_ps = pps.tile([128, BL], FP32, tag="xt_ps")
                nc.tensor.transpose(
                    xt_ps[:], x_sb[:, c * 128 : (c + 1) * 128], ident[0:BL, 0:BL]
                )
                nc.vector.tensor_copy(xt[:, c * BL : (c + 1) * BL], xt_ps[:])

            projs = []
            for nm, w_d, b_d in (
                ("q", wq_d, bq_d),
                ("nk", wk_d, bk_d),
                ("nv", wv_d, bv_d),
            ):
                w_sb = wts.tile([128, 8, F], FP32, tag=f"w_{nm}")
                nc.scalar.dma_start(w_sb[:], w_d.rearrange("(c p) f -> p c f", p=128))
                b_sb = wts.tile([1, F], FP32, tag=f"b_{nm}")
                nc.scalar.dma_start(b_sb[:], b_d[:])
                ps = pps.tile([BL, F], FP32, tag="proj_ps")
                for c in range(8):
                    nc.tensor.matmul(
                        ps[:],
                        xt[:, c * BL : (c + 1) * BL],
                        w_sb[:, c, :],
                        start=(c == 0),
                        stop=False,
                    )
                nc.tensor.matmul(
                    ps[:], ones[0:1, 0:BL], b_sb[:], start=False, stop=True
                )
                r_sb = cst.tile([BL, F], FP32, tag=f"r_{nm}")
                nc.vector.tensor_copy(r_sb[:], ps[:])
                projs.append(r_sb)
            q_sb, nk_sb, nv_sb = projs
            nc.scalar.dma_start(nk_d[:], nk_sb[:])
            nc.scalar.dma_start(nv_d[:], nv_sb[:])

            # qT chunks then block-diagonal qtb [128, 4*CC]
            qt = cst.tile([128, 4 * BL], FP32)
            for c in range(4):
                qt_ps = pps.tile([128, BL], FP32, tag="xt_ps")
                nc.tensor.transpose(
                    qt_ps[:], q_sb[:, c * 128 : (c + 1) * 128], ident[0:BL, 0:BL]
                )
                nc.vector.tensor_copy(qt[:, c * BL : (c + 1) * BL], qt_ps[:])

            qtb = cst.tile([128, 4 * CC], FP32)
            nc.gpsimd.memset(qtb[:], 0.0)
            qtb_v = qtb.rearrange("p (c b h) -> p c b h", c=4, b=BL, h=H)
            qt_v = qt.rearrange("p (c b) -> p c b", c=4)
            for c in range(4):
                for hh in range(2):
                    h = 2 * c + hh
                    nc.vector.tensor_copy(
                        qtb_v[64 * hh : 64 * hh + 64, c, :, h],
                        qt_v[64 * hh : 64 * hh + 64, c, :],
                    )

        # ------------- main loop -------------
        # Per 512-row macro: PE-transpose K into chunk-major KT [f',n],
        # scores [cc, 512] via 4 fat matmuls (stationary = tiny Q blocks),
        # exp once, PE-transpose the weights [cc,128]->[128,cc] per tile,
        # then weights act as stationary for the V product (N=512).
        macros = [("bank", None, j) for j in range(M // RM)]
        macros += [("cache", b, j) for b in range(BL) for j in range(L // RM)]
        with (
            tc.tile_pool(name="acc", bufs=1, space="PSUM") as accp,
            tc.tile_pool(name="mmp", bufs=2, space="PSUM") as mmp,
            tc.tile_pool(name="kv", bufs=3) as kvp,
            tc.tile_pool(name="wrk", bufs=2) as wrk,
        ):
            retr_ps = accp.tile([CC, F], FP32)
            den_ps = accp.tile([CC, 1], FP32)
            zeros = cst.tile([128, H], FP32)
            nc.gpsimd.memset(zeros[:], 0.0)

            for kind, b, j in macros:
                if kind == "bank":
                    ktsrc = mkt_d[j]
                    vsrc = mv_d[j * RM : (j + 1) * RM, :]
                    ncc, coff = CC, 0
                else:
                    ktsrc = pkt_d[b, j]
                    vsrc = pv_d[b, j * RM : (j + 1) * RM, :]
                    ncc, coff = H, b * H
                kt_sb = kvp.tile([128, 4, T, 128], FP32, tag="kt_sb")
                nc.sync.dma_start(
                    kt_sb[:], ktsrc.rearrange("c p t n -> p c t n")
                )
                vmac = kvp.tile([128, T, F], FP32, tag="vmac")
                nc.sync.dma_start(vmac[:], vsrc.rearrange("(t p) f -> p t f", p=128))
                if kind == "cache":
                    kmac = kvp.tile([128, T, F], FP32, tag="kmac")
                    nc.sync.dma_start(
                        kmac[:],
                        pk_d[b, j * RM : (j + 1) * RM, :].rearrange(
                            "(t p) f -> p t f", p=128
                        ),
                    )
                    nc.scalar.dma_start(
                        uk_d[b, j * RM : (j + 1) * RM, :].rearrange(
                            "(t p) f -> p t f", p=128
                        ),
                        kmac[:],
                    )
                    nc.scalar.dma_start(
                        uv_d[b, j * RM : (j + 1) * RM, :].rearrange(
                            "(t p) f -> p t f", p=128
                        ),
                        vmac[:],
                    )
                first = kind == "bank" and j == 0
                last = kind == "cache" and j == (L // RM) - 1

                # scores [ncc, RM] over the whole macro (two 512-wide halves)
                st_ps = mmp.tile([CC, RM], FP32, tag="st_ps")
                for c in range(4):
                    for hf in range(2):
                        nc.tensor.matmul(
                            st_ps[0:ncc, hf * 512 : (hf + 1) * 512],
                            qtb[:, c * CC + coff : c * CC + coff + ncc],
                            kt_sb[:, c, hf * 4 : (hf + 1) * 4, :],
                            start=(c == 0),
                            stop=(c == 3),
                        )
                w_row = wrk.tile([CC, RM], FP32, tag="w_row")
                nc.scalar.activation(w_row[0:ncc, :], st_ps[0:ncc, :], Exp)

                # transpose weights per 128-tile: [ncc,128] -> [128,ncc]
                wt_ps = mmp.tile([128, T * CC], FP32, tag="wt_ps")
                for t in range(T):
                    nc.tensor.transpose(
                        wt_ps[:, t * CC + coff : t * CC + coff + ncc],
                        w_row[0:ncc, t * 128 : (t + 1) * 128],
                        ident[0:ncc, 0:ncc],
                    )
                    if kind == "cache":
                        off = H - coff
                        nc.vector.tensor_copy(
                            wt_ps[:, t * CC + off : t * CC + off + H], zeros[:]
                        )
                wt_sb = wrk.tile([128, T * CC], FP32, tag="wt_sb")
                nc.vector.tensor_copy(wt_sb[:], wt_ps[:])

                st_first = first
                st_last = last
                for t in range(T):
                    nc.tensor.matmul(
                        retr_ps[:],
                        wt_sb[:, t * CC : (t + 1) * CC],
                        vmac[:, t, :],
                        start=(st_first and t == 0),
                        stop=(st_last and t == T - 1),
                        skip_group_check=True,
                    )
                    nc.tensor.matmul(
                        den_ps[:],
                        wt_sb[:, t * CC : (t + 1) * CC],
                        ones[:, 0:1],
                        start=(st_first and t == 0),
                        stop=(st_last and t == T - 1),
                        skip_group_check=True,
                    )

            # normalize while accumulators are live: rn = retr / den
            recp = sb.tile([CC, 1], FP32)
            nc.vector.reciprocal(recp[:], den_ps[:])
            rn = sb.tile([CC, F], FP32)
            nc.vector.tensor_scalar_mul(rn[:], retr_ps[:], recp[:])

        # ------------- epilogue -------------
        with tc.tile_pool(name="eps", bufs=2, space="PSUM") as eps:
            # transpose rn [CC, F] -> rnT chunks [128, CC]
            rnt = sb.tile([128, 4 * CC], FP32)
            for c in range(4):
                rnt_ps = eps.tile([128, CC], FP32, tag="rnt_ps")
                nc.tensor.transpose(
                    rnt_ps[:], rn[:, c * 128 : (c + 1) * 128], ident[0:CC, 0:CC]
                )
                nc.vector.tensor_copy(rnt[:, c * CC : (c + 1) * CC], rnt_ps[:])

            # extract diagonal head blocks: rtn[(c f'), b] = rnT[(c f'), b*8+h(f')]
            rtn = sb.tile([128, 4 * BL], FP32)
            rtn_v = rtn.rearrange("p (c b) -> p c b", c=4)
            rnt_v = rnt.rearrange("p (c b h) -> p c b h", c=4, b=BL, h=H)
            for c in range(4):
                for hh in range(2):
                    h = 2 * c + hh
                    sl = slice(64 * hh, 64 * hh + 64)
                    nc.vector.tensor_copy(
                        rtn_v[sl, c, :], rnt_v[sl, c, :, h]
                    )

            o_sb = sb.tile([BL, D], FP32)
            for half in range(2):
                op_ps = eps.tile([BL, 512], FP32, tag="op")
                for c in range(4):
                    nc.tensor.matmul(
                        op_ps[:],
                        rtn_v[:, c, :],
                        wo_sb[:, c, half * 512 : (half + 1) * 512],
                        start=(c == 0),
                        stop=False,
                    )
                nc.tensor.matmul(
                    op_ps[:],
                    ones[0:1, 0:BL],
                    bo_sb[0:1, half * 512 : (half + 1) * 512],
                    start=False,
                    stop=True,
                )
                nc.vector.tensor_copy(o_sb[:, half * 512 : (half + 1) * 512], op_ps[:])
            nc.sync.dma_start(out_d[:], o_sb[:])


_NC_CACHE = {}


def _get_nc():
    if "nc" not in _NC_CACHE:
        nc = bacc.Bacc(
            "TRN2", target_bir_lowering=False, debug=False, num_devices=NCORES
        )
        with tile.TileContext(nc) as tc:
            _emit(nc, tc)
        nc.compile()
        _NC_CACHE["nc"] = nc
    return _NC_CACHE["nc"]


def _make_in_maps(
    x, prev_keys, prev_values, Wq, bq, Wk, bk, Wv, bv, Wo, bo,
    memory_keys, memory_values,
):
    f32 = np.float32
    x = np.ascontiguousarray(np.asarray(x, f32))
    pk = np.asarray(prev_keys, f32).reshape(B, L, F)
    pv = np.asarray(prev_values, f32).reshape(B, L, F)
    scale = np.float32(1.0 / np.sqrt(KD))
    rep = {
        "wq": np.ascontiguousarray(np.asarray(Wq, f32) * scale),
        "bq": np.asarray(bq, f32).reshape(1, F) * scale,
        "wk": np.ascontiguousarray(np.asarray(Wk, f32)),
        "bk": np.asarray(bk, f32).reshape(1, F),
        "wv": np.ascontiguousarray(np.asarray(Wv, f32)),
        "bv": np.asarray(bv, f32).reshape(1, F),
        "wo": np.ascontiguousarray(np.asarray(Wo, f32)),
        "bo": np.asarray(bo, f32).reshape(1, D),
        "mkt": np.ascontiguousarray(
            np.asarray(memory_keys, f32)
            .reshape(M // RM, T, 128, 4, 128)
            .transpose(0, 3, 4, 1, 2)
        ),
        "mv": np.ascontiguousarray(np.asarray(memory_values, f32)),
        "ident": np.eye(128, dtype=f32),
        "ones": np.ones((128, 128), f32),
    }
    pkt = np.ascontiguousarray(
        pk.reshape(B, L // RM, T, 128, 4, 128).transpose(0, 1, 4, 5, 2, 3)
    )
    in_maps = []
    for i in range(NCORES):
        m = dict(rep)
        m["x"] = x[i * BL : (i + 1) * BL]
        m["pk"] = pk[i * BL : (i + 1) * BL]
        m["pkt"] = pkt[i * BL : (i + 1) * BL]
        m["pv"] = pv[i * BL : (i + 1) * BL]
        in_maps.append(m)
    return in_maps


def _assemble(results):
    f32 = np.float32
    output = np.concatenate([np.asarray(r["out"]) for r in results], axis=0)
    upd_k = np.empty((B, L + 1, H, KD), f32)
    upd_v = np.empty((B, L + 1, H, VD), f32)
    for i in range(NCORES):
        bs = slice(i * BL, (i + 1) * BL)
        upd_k[bs, 0:L] = np.asarray(results[i]["uk"]).reshape(BL, L, H, KD)
        upd_v[bs, 0:L] = np.asarray(results[i]["uv"]).reshape(BL, L, H, VD)
        upd_k[bs, L] = np.asarray(results[i]["newk"]).reshape(BL, H, KD)
        upd_v[bs, L] = np.asarray(results[i]["newv"]).reshape(BL, H, VD)
    return output, upd_k, upd_v


def kernel(**inputs):
    in_maps = _make_in_maps(**inputs)
    nc = _get_nc()
    res = run_bass_kernel_spmd(nc, in_maps, list(range(NCORES)))
    _NC_CACHE["last_results"] = res
    return _assemble(res.results)
